# revision 15
# baseline (speedup 1.0000x reference)
"""Trainium2 Bass kernel for nn_Att_AdaIn (B=4, C=256, H=W=64 attention block).

Sharding: 8 cores = 4 batches x 2 query-halves. Each core holds the fused
weights, the full key/value source y[b] ([256, 4096]), and its own query
slice x[b][:, half] ([256, 2048]); it computes the full attention output for
its 2048 queries. Host gathers the 8 [256, 2048] results.

Weight fusion done on the host (in float64):
  logits: S = k^T q with q = Wq x + bq, k = Wk y + bk
        = y^T (Wk^T Wq) x + y^T (Wk^T bq) 1^T + [per-query-constant terms]
    The per-query-constant (l-only) terms are softmax-invariant and dropped.
    So with  M^T = (Wk^T Wq)^T  and  bw = Wk^T bq:   qm = M x + bw,
    ST[j,l] = sum_c y[c,j] qm[c,l].
  output: Wo (V E / den) + bo  with V = Wv y + bv 1^T
        = (Wo Wv) y E / den + Wo bv + bo
    So with MoT = (Wo Wv)^T and bo2 = bo + Wo bv, the value projection
    vTo = y^T MoT directly produces Wo-mixed values and the separate
    output projection disappears.

Per-core pipeline (layouts chosen so no on-chip transpose is needed):
  qm  = M x + bw               [c, l]      (c on partitions)
  vToa= y^T [MoT | 1col | 0]   [j, 258]    (j on partitions; col 256 == 1 via
                                           broadcast-bias add -> softmax
                                           denominators come out of the same
                                           matmuls as the values)
  ST  = y^T qm                 [j, l]      (transposed attention scores)
  E   = exp(ST / sqrt(C))      (no max-subtraction: logits ~ N(0,1), fp32-safe)
  zA  = vToa^T E               [258, l]    rows 0..255 = unnormalized Wo-mixed
                                           output, row 256 = denominator
  out = zA * (1/den) + bo2 + x

Matmul dtype selectable: float32 (4 cyc/row), float32r (~2 cyc/row on HW),
bfloat16 (1 cyc/row, FWL). For float32r/bfloat16 every tile feeding a matmul
is typed in that dtype (BIR requires producers to round).
"""

import os
import sys

for _p in ("/root/.axon_site", "/root/.axon_site/_ro/trn_rl_repo", "/opt/trn_rl_repo"):
    if os.path.isdir(_p) and _p not in sys.path:
        sys.path.append(_p)

import numpy as np

import concourse.bass as bass
from concourse import bacc, mybir, tile
from concourse import bass_utils

B, C, H, W = 4, 256, 64, 64
N = H * W          # 4096 pixels
NQ = N // 2        # 2048 queries per core
P = 128
A = C // P         # 2 channel chunks
LT = 512           # l-tile (query) width
NLT = NQ // LT     # 4 l-tiles
JC = N // P        # 32 key chunks
SCALE = 1.0 / np.sqrt(np.float32(C))  # 1/16
CP = C             # value projection width (Wo-mixed channels)

MATMUL_DT = os.environ.get("ATT_MATMUL_DT", "bfloat16")


def build_nc(matmul_dt_name: str = MATMUL_DT):
    mdt = getattr(mybir.dt, matmul_dt_name)
    f32 = mybir.dt.float32
    is_bf16 = mdt == mybir.dt.bfloat16

    nc = bacc.Bacc("TRN2", target_bir_lowering=False, debug=False)

    x_d = nc.dram_tensor("x", [C, NQ], mdt, kind="ExternalInput").ap()
    y_d = nc.dram_tensor("y", [C, N], mdt, kind="ExternalInput").ap()
    mT_d = nc.dram_tensor("mT", [C, C], mdt, kind="ExternalInput").ap()
    moTa_d = nc.dram_tensor("moTa", [C, CP], mdt, kind="ExternalInput").ap()
    if is_bf16:
        xres_d = nc.dram_tensor("xres", [C, NQ], f32, kind="ExternalInput").ap()
    bw_d = nc.dram_tensor("bw", [C], f32, kind="ExternalInput").ap()
    bo2_d = nc.dram_tensor("bo2", [C], f32, kind="ExternalInput").ap()
    out_d = nc.dram_tensor("out", [C, NQ], f32, kind="ExternalOutput").ap()

    xr = x_d.rearrange("(a p) n -> p a n", p=P)
    yr = y_d.rearrange("(a p) n -> p a n", p=P)
    outr = out_d.rearrange("(a p) n -> p a n", p=P)

    with tile.TileContext(nc) as tc:
        with (
            tc.tile_pool(name="const", bufs=1) as const,
            tc.tile_pool(name="epool", bufs=6) as epool,
            tc.tile_pool(name="opool", bufs=3) as opool,
            tc.tile_pool(name="rpool", bufs=2) as rpool,
            tc.tile_pool(name="ps_st", bufs=3, space="PSUM") as ps_st,
            tc.tile_pool(name="ps_zq", bufs=2, space="PSUM") as ps_zq,
            tc.tile_pool(name="ps_den", bufs=1, space="PSUM") as ps_den,
            tc.tile_pool(name="dpool", bufs=2, space="DRAM") as dpool,
        ):
            # ---- persistent SBUF tensors ----
            x_sb = const.tile([P, A, NQ], mdt)
            y_sb = const.tile([P, A, N], mdt)
            mT_sb = const.tile([P, A, C], mdt)
            moTa_sb = const.tile([P, A, CP], mdt)
            bw_sb = const.tile([P, A], f32)
            bo2_sb = const.tile([P, A], f32)
            ones_col = const.tile([P, 1], mdt)
            qm_sb = const.tile([P, A, NQ], mdt)
            vTo_sb = const.tile([P, JC, CP], mdt)
            if is_bf16:
                xres_sb = const.tile([P, A, NQ], f32)
            else:
                xres_sb = x_sb.bitcast(f32)

            # ---- loads ----
            nc.sync.dma_start(out=x_sb, in_=xr)
            nc.sync.dma_start(out=mT_sb, in_=mT_d.rearrange("(a p) o -> p a o", p=P))
            nc.sync.dma_start(out=y_sb, in_=yr)
            nc.gpsimd.dma_start(out=moTa_sb, in_=moTa_d.rearrange("(a p) o -> p a o", p=P))
            nc.gpsimd.dma_start(out=bw_sb, in_=bw_d.rearrange("(a p) -> p a", p=P))
            nc.gpsimd.dma_start(out=bo2_sb, in_=bo2_d.rearrange("(a p) -> p a", p=P))
            nc.vector.memset(ones_col, 1.0)
            if is_bf16:
                nc.scalar.dma_start(
                    out=xres_sb, in_=xres_d.rearrange("(a p) n -> p a n", p=P)
                )

            # ---- projections ----
            # qm[c, l] = sum_c' M[c, c'] x[c', l] + bw[c]
            for och in range(A):
                for lt in range(NLT):
                    ps = ps_st.tile([P, LT], f32, tag="st")
                    for a in range(A):
                        nc.tensor.matmul(
                            ps,
                            mT_sb[:, a, och * P:(och + 1) * P],
                            x_sb[:, a, lt * LT:(lt + 1) * LT],
                            start=(a == 0),
                            stop=(a == A - 1),
                        )
                    nc.scalar.activation(
                        out=qm_sb[:, och, lt * LT:(lt + 1) * LT],
                        in_=ps,
                        func=mybir.ActivationFunctionType.Identity,
                        bias=bw_sb[:, och:och + 1],
                    )
            # vTo[j, o] = sum_c y[c, j] MoTa[c, o] + bva[o]   (o in 0..258)
            for jc in range(JC):
                ps = ps_st.tile([P, CP], f32, tag="st")
                for a in range(A):
                    nc.tensor.matmul(
                        ps,
                        y_sb[:, a, jc * P:(jc + 1) * P],
                        moTa_sb[:, a, :],
                        start=(a == 0),
                        stop=(a == A - 1),
                    )
                nc.scalar.activation(
                    out=vTo_sb[:, jc, :],
                    in_=ps,
                    func=mybir.ActivationFunctionType.Copy,
                )

            # ---- attention, l-tile at a time ----
            for lt in range(NLT):
                lsl = slice(lt * LT, (lt + 1) * LT)
                zq0 = ps_zq.tile([P, LT], f32, tag="zq0")
                zq1 = ps_zq.tile([P, LT], f32, tag="zq1")
                zq = (zq0, zq1)
                eacc = epool.tile([P, LT], mdt, tag="eacc")
                for jc in range(JC):
                    st = ps_st.tile([P, LT], f32, tag="st")
                    for a in range(A):
                        nc.tensor.matmul(
                            st,
                            y_sb[:, a, jc * P:(jc + 1) * P],
                            qm_sb[:, a, lsl],
                            start=(a == 0),
                            stop=(a == A - 1),
                        )
                    e_sb = epool.tile([P, LT], mdt)
                    nc.scalar.activation(
                        out=e_sb,
                        in_=st,
                        func=mybir.ActivationFunctionType.Exp,
                        scale=float(SCALE),
                    )
                    for m in range(A):
                        nc.tensor.matmul(
                            zq[m],
                            vTo_sb[:, jc, m * P:(m + 1) * P],
                            e_sb,
                            start=(jc == 0),
                            stop=(jc == JC - 1),
                        )
                    if jc == 0:
                        nc.vector.tensor_copy(out=eacc, in_=e_sb)
                    else:
                        nc.vector.tensor_add(out=eacc, in0=eacc, in1=e_sb)

                # reduce E over partitions -> denominators, then 1/den,
                # broadcast back across partitions via a DRAM round-trip
                den = ps_den.tile([1, LT], f32)
                nc.tensor.matmul(den, ones_col, eacc, start=True, stop=True)
                r_sb = rpool.tile([1, LT], f32, tag="r")
                nc.vector.reciprocal(out=r_sb, in_=den)
                r_dram = dpool.tile([1, LT], f32, tag="rdram")
                nc.sync.dma_start(out=r_dram, in_=r_sb)
                rbc_sb = rpool.tile([P, LT], f32, tag="rbc")
                r_bcast_ap = bass.AP(
                    tensor=r_dram.tensor,
                    offset=r_dram.offset,
                    ap=[[0, P], list(r_dram.ap[-1])],
                )
                nc.sync.dma_start(out=rbc_sb, in_=r_bcast_ap)

                # out = zq * r + bo2 + x
                for och in range(A):
                    o_sb = opool.tile([P, LT], f32)
                    nc.vector.tensor_mul(out=o_sb, in0=zq[och], in1=rbc_sb)
                    nc.vector.tensor_scalar_add(
                        out=o_sb, in0=o_sb, scalar1=bo2_sb[:, och:och + 1]
                    )
                    nc.vector.tensor_add(out=o_sb, in0=o_sb, in1=xres_sb[:, och, lsl])
                    nc.sync.dma_start(out=outr[:, och, lsl], in_=o_sb)

    nc.compile()
    return nc


_NC_CACHE = {}


def _get_nc(matmul_dt_name: str = MATMUL_DT):
    if matmul_dt_name not in _NC_CACHE:
        _NC_CACHE[matmul_dt_name] = build_nc(matmul_dt_name)
    return _NC_CACHE[matmul_dt_name]


def make_in_maps(x, y, Wq, bq, Wk, bk, Wv, bv, Wo, bo, matmul_dt_name: str = MATMUL_DT):
    f32 = np.float32
    f64 = np.float64
    if matmul_dt_name == "bfloat16":
        import ml_dtypes

        mnp = ml_dtypes.bfloat16
    else:
        mnp = np.float32
    xf = np.asarray(x, f32).reshape(B, C, N)
    yf = np.asarray(y, f32).reshape(B, C, N)
    Wq64, Wk64, Wv64, Wo64 = (np.asarray(w, f64) for w in (Wq, Wk, Wv, Wo))
    bq64, bv64, bo64 = (np.asarray(b, f64) for b in (bq, bv, bo))
    mT = np.ascontiguousarray((Wk64.T @ Wq64).T).astype(mnp)   # [c_x, c_qm]... (M^T)
    moTa = np.ascontiguousarray((Wo64 @ Wv64).T).astype(mnp)   # [c_y, o]
    bw = (Wk64.T @ bq64).astype(f32)
    bo2 = (bo64 + Wo64 @ bv64).astype(f32)
    in_maps = []
    for core in range(8):
        b, h = divmod(core, 2)
        xs = np.ascontiguousarray(xf[b][:, h * NQ:(h + 1) * NQ])
        m = {
            "x": xs.astype(mnp) if mnp is not np.float32 else xs,
            "y": yf[b].astype(mnp) if mnp is not np.float32 else yf[b],
            "mT": mT, "moTa": moTa,
            "bw": bw, "bo2": bo2,
        }
        if matmul_dt_name == "bfloat16":
            m["xres"] = xs
        in_maps.append(m)
    return in_maps


def kernel(x, y, Wq, bq, Wk, bk, Wv, bv, Wo, bo):
    nc = _get_nc()
    in_maps = make_in_maps(x, y, Wq, bq, Wk, bk, Wv, bv, Wo, bo)
    res = bass_utils.run_bass_kernel_spmd(nc, in_maps, core_ids=list(range(8)))
    out = np.empty((B, C, N), np.float32)
    for core in range(8):
        b, h = divmod(core, 2)
        out[b][:, h * NQ:(h + 1) * NQ] = res.results[core]["out"]
    return out.reshape(B, C, H, W)


# revision 17
# speedup vs baseline: 1.1968x; 1.1968x over previous
"""Trainium2 Bass kernel for nn_Att_AdaIn (B=4, C=256, H=W=64 attention block).

Sharding: 8 cores = 4 batches x 2 query-halves. Each core holds the fused
weights, the full key/value source y[b] ([256, 4096]), and its own query
slice x[b][:, half] ([256, 2048]); it computes the full attention output for
its 2048 queries. Host gathers the 8 [256, 2048] results.

Weight fusion done on the host (in float64):
  logits: S = k^T q with q = Wq x + bq, k = Wk y + bk
        = y^T (Wk^T Wq) x + y^T (Wk^T bq) 1^T + [per-query-constant terms]
    The per-query-constant (l-only) terms are softmax-invariant and dropped.
    So with  M^T = (Wk^T Wq)^T  and  bw = Wk^T bq:   qm = M x + bw,
    ST[j,l] = sum_c y[c,j] qm[c,l].
  output: Wo (V E / den) + bo  with V = Wv y + bv 1^T
        = (Wo Wv) y E / den + Wo bv + bo
    So with MoT = (Wo Wv)^T and bo2 = bo + Wo bv, the value projection
    vTo = y^T MoT directly produces Wo-mixed values and the separate
    output projection disappears.

Per-core pipeline (layouts chosen so no on-chip transpose is needed):
  qm  = M x + bw               [c, l]      (c on partitions)
  vToa= y^T [MoT | 1col | 0]   [j, 258]    (j on partitions; col 256 == 1 via
                                           broadcast-bias add -> softmax
                                           denominators come out of the same
                                           matmuls as the values)
  ST  = y^T qm                 [j, l]      (transposed attention scores)
  E   = exp(ST / sqrt(C))      (no max-subtraction: logits ~ N(0,1), fp32-safe)
  zA  = vToa^T E               [258, l]    rows 0..255 = unnormalized Wo-mixed
                                           output, row 256 = denominator
  out = zA * (1/den) + bo2 + x

Matmul dtype selectable: float32 (4 cyc/row), float32r (~2 cyc/row on HW),
bfloat16 (1 cyc/row, FWL). For float32r/bfloat16 every tile feeding a matmul
is typed in that dtype (BIR requires producers to round).
"""

import os
import sys

for _p in ("/root/.axon_site", "/root/.axon_site/_ro/trn_rl_repo", "/opt/trn_rl_repo"):
    if os.path.isdir(_p) and _p not in sys.path:
        sys.path.append(_p)

import numpy as np

import concourse.bass as bass
from concourse import bacc, mybir, tile
from concourse import bass_utils

B, C, H, W = 4, 256, 64, 64
N = H * W          # 4096 pixels
NQ = N // 2        # 2048 queries per core
P = 128
A = C // P         # 2 channel chunks
LT = 512           # l-tile (query) width
NLT = NQ // LT     # 4 l-tiles
JC = N // P        # 32 key chunks
SCALE = 1.0 / np.sqrt(np.float32(C))  # 1/16
CP = C             # value projection width (Wo-mixed channels)

MATMUL_DT = os.environ.get("ATT_MATMUL_DT", "bfloat16")


def build_nc(matmul_dt_name: str = MATMUL_DT):
    mdt = getattr(mybir.dt, matmul_dt_name)
    f32 = mybir.dt.float32
    is_bf16 = mdt == mybir.dt.bfloat16

    nc = bacc.Bacc("TRN2", target_bir_lowering=False, debug=False)

    x_d = nc.dram_tensor("x", [C, NQ], mdt, kind="ExternalInput").ap()
    y_d = nc.dram_tensor("y", [C, N], mdt, kind="ExternalInput").ap()
    mT_d = nc.dram_tensor("mT", [C, C], mdt, kind="ExternalInput").ap()
    moTa_d = nc.dram_tensor("moTa", [C, CP], mdt, kind="ExternalInput").ap()
    if is_bf16:
        xres_d = nc.dram_tensor("xres", [C, NQ], f32, kind="ExternalInput").ap()
    bw_d = nc.dram_tensor("bw", [C], f32, kind="ExternalInput").ap()
    bo2_d = nc.dram_tensor("bo2", [C], f32, kind="ExternalInput").ap()
    out_d = nc.dram_tensor("out", [C, NQ], f32, kind="ExternalOutput").ap()

    xr = x_d.rearrange("(a p) n -> p a n", p=P)
    yr = y_d.rearrange("(a p) n -> p a n", p=P)
    outr = out_d.rearrange("(a p) n -> p a n", p=P)

    with tile.TileContext(nc) as tc:
        with (
            tc.tile_pool(name="const", bufs=1) as const,
            tc.tile_pool(name="epool", bufs=8) as epool,
            tc.tile_pool(name="opool", bufs=3) as opool,
            tc.tile_pool(name="rpool", bufs=2) as rpool,
            tc.tile_pool(name="ps_st", bufs=3, space="PSUM") as ps_st,
            tc.tile_pool(name="ps_zq", bufs=2, space="PSUM") as ps_zq,
            tc.tile_pool(name="ps_den", bufs=1, space="PSUM") as ps_den,
            tc.tile_pool(name="dpool", bufs=2, space="DRAM") as dpool,
        ):
            # ---- persistent SBUF tensors ----
            x_sb = const.tile([P, A, NQ], mdt)
            y_sb = const.tile([P, A, N], mdt)
            mT_sb = const.tile([P, A, C], mdt)
            moTa_sb = const.tile([P, A, CP], mdt)
            bw_sb = const.tile([P, A], f32)
            bo2_sb = const.tile([P, A], f32)
            ones_col = const.tile([P, 1], mdt)
            qm_sb = const.tile([P, A, NQ], mdt)
            vTo_sb = const.tile([P, JC, CP], mdt)
            if is_bf16:
                xres_sb = const.tile([P, A, NQ], f32)
            else:
                xres_sb = x_sb.bitcast(f32)

            # ---- loads ----
            nc.sync.dma_start(out=x_sb, in_=xr)
            nc.sync.dma_start(out=mT_sb, in_=mT_d.rearrange("(a p) o -> p a o", p=P))
            nc.sync.dma_start(out=y_sb, in_=yr)
            nc.sync.dma_start(out=moTa_sb, in_=moTa_d.rearrange("(a p) o -> p a o", p=P))
            nc.sync.dma_start(out=bw_sb, in_=bw_d.rearrange("(a p) -> p a", p=P))
            nc.sync.dma_start(out=bo2_sb, in_=bo2_d.rearrange("(a p) -> p a", p=P))
            nc.vector.memset(ones_col, 1.0)
            if is_bf16:
                nc.sync.dma_start(
                    out=xres_sb, in_=xres_d.rearrange("(a p) n -> p a n", p=P)
                )

            # ---- projections ----
            # qm[c, l] = sum_c' M[c, c'] x[c', l] + bw[c]
            for och in range(A):
                for lt in range(NLT):
                    ps = ps_st.tile([P, LT], f32, tag="st")
                    for a in range(A):
                        nc.tensor.matmul(
                            ps,
                            mT_sb[:, a, och * P:(och + 1) * P],
                            x_sb[:, a, lt * LT:(lt + 1) * LT],
                            start=(a == 0),
                            stop=(a == A - 1),
                        )
                    nc.scalar.activation(
                        out=qm_sb[:, och, lt * LT:(lt + 1) * LT],
                        in_=ps,
                        func=mybir.ActivationFunctionType.Identity,
                        bias=bw_sb[:, och:och + 1],
                    )
            # vTo[j, o] = sum_c y[c, j] MoTa[c, o] + bva[o]   (o in 0..258)
            for jc in range(JC):
                ps = ps_st.tile([P, CP], f32, tag="st")
                for a in range(A):
                    nc.tensor.matmul(
                        ps,
                        y_sb[:, a, jc * P:(jc + 1) * P],
                        moTa_sb[:, a, :],
                        start=(a == 0),
                        stop=(a == A - 1),
                    )
                nc.scalar.activation(
                    out=vTo_sb[:, jc, :],
                    in_=ps,
                    func=mybir.ActivationFunctionType.Copy,
                )

            # ---- attention, l-tile at a time ----
            for lt in range(NLT):
                lsl = slice(lt * LT, (lt + 1) * LT)
                zq0 = ps_zq.tile([P, LT], f32, tag="zq0")
                zq1 = ps_zq.tile([P, LT], f32, tag="zq1")
                zq = (zq0, zq1)
                eacc = epool.tile([P, LT], mdt, tag="eacc")
                for jc in range(JC):
                    st = ps_st.tile([P, LT], f32, tag="st")
                    for a in range(A):
                        nc.tensor.matmul(
                            st,
                            y_sb[:, a, jc * P:(jc + 1) * P],
                            qm_sb[:, a, lsl],
                            start=(a == 0),
                            stop=(a == A - 1),
                        )
                    e_sb = epool.tile([P, LT], mdt)
                    nc.scalar.activation(
                        out=e_sb,
                        in_=st,
                        func=mybir.ActivationFunctionType.Exp,
                        scale=float(SCALE),
                    )
                    for m in range(A):
                        nc.tensor.matmul(
                            zq[m],
                            vTo_sb[:, jc, m * P:(m + 1) * P],
                            e_sb,
                            start=(jc == 0),
                            stop=(jc == JC - 1),
                        )
                    if jc == 0:
                        nc.vector.tensor_copy(out=eacc, in_=e_sb)
                    else:
                        nc.vector.tensor_add(out=eacc, in0=eacc, in1=e_sb)

                # reduce E over partitions -> denominators, then 1/den,
                # broadcast back across partitions via a DRAM round-trip
                den = ps_den.tile([1, LT], f32)
                nc.tensor.matmul(den, ones_col, eacc, start=True, stop=True)
                r_sb = rpool.tile([1, LT], f32, tag="r")
                nc.vector.reciprocal_approx_fast(out=r_sb, in_=den)
                r_dram = dpool.tile([1, LT], f32, tag="rdram")
                nc.sync.dma_start(out=r_dram, in_=r_sb)
                rbc_sb = rpool.tile([P, LT], f32, tag="rbc")
                r_bcast_ap = bass.AP(
                    tensor=r_dram.tensor,
                    offset=r_dram.offset,
                    ap=[[0, P], list(r_dram.ap[-1])],
                )
                nc.sync.dma_start(out=rbc_sb, in_=r_bcast_ap)

                # out = zq * r + bo2 + x
                for och in range(A):
                    o_sb = opool.tile([P, LT], f32)
                    nc.vector.tensor_mul(out=o_sb, in0=zq[och], in1=rbc_sb)
                    nc.vector.tensor_scalar_add(
                        out=o_sb, in0=o_sb, scalar1=bo2_sb[:, och:och + 1]
                    )
                    nc.vector.tensor_add(out=o_sb, in0=o_sb, in1=xres_sb[:, och, lsl])
                    nc.sync.dma_start(out=outr[:, och, lsl], in_=o_sb)

    nc.compile()
    return nc


_NC_CACHE = {}


def _get_nc(matmul_dt_name: str = MATMUL_DT):
    if matmul_dt_name not in _NC_CACHE:
        _NC_CACHE[matmul_dt_name] = build_nc(matmul_dt_name)
    return _NC_CACHE[matmul_dt_name]


def make_in_maps(x, y, Wq, bq, Wk, bk, Wv, bv, Wo, bo, matmul_dt_name: str = MATMUL_DT):
    f32 = np.float32
    f64 = np.float64
    if matmul_dt_name == "bfloat16":
        import ml_dtypes

        mnp = ml_dtypes.bfloat16
    else:
        mnp = np.float32
    xf = np.asarray(x, f32).reshape(B, C, N)
    yf = np.asarray(y, f32).reshape(B, C, N)
    Wq64, Wk64, Wv64, Wo64 = (np.asarray(w, f64) for w in (Wq, Wk, Wv, Wo))
    bq64, bv64, bo64 = (np.asarray(b, f64) for b in (bq, bv, bo))
    mT = np.ascontiguousarray((Wk64.T @ Wq64).T).astype(mnp)   # [c_x, c_qm]... (M^T)
    moTa = np.ascontiguousarray((Wo64 @ Wv64).T).astype(mnp)   # [c_y, o]
    bw = (Wk64.T @ bq64).astype(f32)
    bo2 = (bo64 + Wo64 @ bv64).astype(f32)
    in_maps = []
    for core in range(8):
        b, h = divmod(core, 2)
        xs = np.ascontiguousarray(xf[b][:, h * NQ:(h + 1) * NQ])
        m = {
            "x": xs.astype(mnp) if mnp is not np.float32 else xs,
            "y": yf[b].astype(mnp) if mnp is not np.float32 else yf[b],
            "mT": mT, "moTa": moTa,
            "bw": bw, "bo2": bo2,
        }
        if matmul_dt_name == "bfloat16":
            m["xres"] = xs
        in_maps.append(m)
    return in_maps


def kernel(x, y, Wq, bq, Wk, bk, Wv, bv, Wo, bo):
    nc = _get_nc()
    in_maps = make_in_maps(x, y, Wq, bq, Wk, bk, Wv, bv, Wo, bo)
    res = bass_utils.run_bass_kernel_spmd(nc, in_maps, core_ids=list(range(8)))
    out = np.empty((B, C, N), np.float32)
    for core in range(8):
        b, h = divmod(core, 2)
        out[b][:, h * NQ:(h + 1) * NQ] = res.results[core]["out"]
    return out.reshape(B, C, H, W)


# revision 18
# speedup vs baseline: 1.3904x; 1.1618x over previous
"""Trainium2 Bass kernel for nn_Att_AdaIn (B=4, C=256, H=W=64 attention block).

Sharding: 8 cores = 4 batches x 2 query-halves. Each core holds the fused
weights, the full key/value source y[b] ([256, 4096]), and its own query
slice x[b][:, half] ([256, 2048]); it computes the full attention output for
its 2048 queries. Host gathers the 8 [256, 2048] results.

Weight fusion done on the host (in float64):
  logits: S = k^T q with q = Wq x + bq, k = Wk y + bk
        = y^T (Wk^T Wq) x + y^T (Wk^T bq) 1^T + [per-query-constant terms]
    The per-query-constant (l-only) terms are softmax-invariant and dropped.
    So with  M^T = (Wk^T Wq)^T  and  bw = Wk^T bq:   qm = M x + bw,
    ST[j,l] = sum_c y[c,j] qm[c,l].
  output: Wo (V E / den) + bo  with V = Wv y + bv 1^T
        = (Wo Wv) y E / den + Wo bv + bo
    So with MoT = (Wo Wv)^T and bo2 = bo + Wo bv, the value projection
    vTo = y^T MoT directly produces Wo-mixed values and the separate
    output projection disappears.

Per-core pipeline (layouts chosen so no on-chip transpose is needed):
  qm  = M x + bw               [c, l]      (c on partitions)
  vToa= y^T [MoT | 1col | 0]   [j, 258]    (j on partitions; col 256 == 1 via
                                           broadcast-bias add -> softmax
                                           denominators come out of the same
                                           matmuls as the values)
  ST  = y^T qm                 [j, l]      (transposed attention scores)
  E   = exp(ST / sqrt(C))      (no max-subtraction: logits ~ N(0,1), fp32-safe)
  zA  = vToa^T E               [258, l]    rows 0..255 = unnormalized Wo-mixed
                                           output, row 256 = denominator
  out = zA * (1/den) + bo2 + x

Matmul dtype selectable: float32 (4 cyc/row), float32r (~2 cyc/row on HW),
bfloat16 (1 cyc/row, FWL). For float32r/bfloat16 every tile feeding a matmul
is typed in that dtype (BIR requires producers to round).
"""

import os
import sys

for _p in ("/root/.axon_site", "/root/.axon_site/_ro/trn_rl_repo", "/opt/trn_rl_repo"):
    if os.path.isdir(_p) and _p not in sys.path:
        sys.path.append(_p)

import numpy as np

import concourse.bass as bass
from concourse import bacc, mybir, tile
from concourse import bass_utils

B, C, H, W = 4, 256, 64, 64
N = H * W          # 4096 pixels
NQ = N // 2        # 2048 queries per core
P = 128
A = C // P         # 2 channel chunks
LT = 512           # l-tile (query) width
NLT = NQ // LT     # 4 l-tiles
JC = N // P        # 32 key chunks
SCALE = 1.0 / np.sqrt(np.float32(C))  # 1/16
CP = C             # value projection width (Wo-mixed channels)

MATMUL_DT = os.environ.get("ATT_MATMUL_DT", "bfloat16")
ST_FP8 = os.environ.get("ATT_ST_FP8", "0") == "1"


def build_nc(matmul_dt_name: str = MATMUL_DT):
    mdt = getattr(mybir.dt, matmul_dt_name)
    f32 = mybir.dt.float32
    is_bf16 = mdt == mybir.dt.bfloat16

    nc = bacc.Bacc("TRN2", target_bir_lowering=False, debug=False)

    x_d = nc.dram_tensor("x", [C, NQ], mdt, kind="ExternalInput").ap()
    y_d = nc.dram_tensor("y", [C, N], mdt, kind="ExternalInput").ap()
    mT_d = nc.dram_tensor("mT", [C, C], mdt, kind="ExternalInput").ap()
    if ST_FP8:
        y8_d = nc.dram_tensor("y8", [C, N], mybir.dt.float8e4, kind="ExternalInput").ap()
    moTa_d = nc.dram_tensor("moTa", [C, CP], mdt, kind="ExternalInput").ap()
    if is_bf16:
        xres_d = nc.dram_tensor("xres", [C, NQ], f32, kind="ExternalInput").ap()
    bw_d = nc.dram_tensor("bw", [C], f32, kind="ExternalInput").ap()
    bo2_d = nc.dram_tensor("bo2", [C], f32, kind="ExternalInput").ap()
    out_d = nc.dram_tensor("out", [C, NQ], f32, kind="ExternalOutput").ap()

    xr = x_d.rearrange("(a p) n -> p a n", p=P)
    yr = y_d.rearrange("(a p) n -> p a n", p=P)
    outr = out_d.rearrange("(a p) n -> p a n", p=P)

    with tile.TileContext(nc) as tc:
        with (
            tc.tile_pool(name="const", bufs=1) as const,
            tc.tile_pool(name="epool", bufs=8) as epool,
            tc.tile_pool(name="opool", bufs=3) as opool,
            tc.tile_pool(name="rpool", bufs=2) as rpool,
            tc.tile_pool(name="ps_st", bufs=3, space="PSUM") as ps_st,
            tc.tile_pool(name="ps_zq", bufs=2, space="PSUM") as ps_zq,
            tc.tile_pool(name="ps_den", bufs=1, space="PSUM") as ps_den,
            tc.tile_pool(name="dpool", bufs=2, space="DRAM") as dpool,
        ):
            # ---- persistent SBUF tensors ----
            x_sb = const.tile([P, A, NQ], mdt)
            y_sb = const.tile([P, A, N], mdt)
            mT_sb = const.tile([P, A, C], mdt)
            moTa_sb = const.tile([P, A, CP], mdt)
            bw_sb = const.tile([P, A], f32)
            bo2_sb = const.tile([P, A], f32)
            ones_col = const.tile([P, 1], mdt)
            qm_dt = mybir.dt.float8e4 if ST_FP8 else mdt
            qm_sb = const.tile([P, A, NQ], qm_dt)
            if ST_FP8:
                y8_sb = const.tile([P, A, N], mybir.dt.float8e4)
            vTo_sb = const.tile([P, JC, CP], mdt)
            if is_bf16:
                xres_sb = const.tile([P, A, NQ], f32)
            else:
                xres_sb = x_sb.bitcast(f32)

            # ---- loads ----
            nc.sync.dma_start(out=x_sb, in_=xr)
            nc.sync.dma_start(out=mT_sb, in_=mT_d.rearrange("(a p) o -> p a o", p=P))
            nc.sync.dma_start(out=y_sb, in_=yr)
            if ST_FP8:
                nc.sync.dma_start(out=y8_sb, in_=y8_d.rearrange("(a p) n -> p a n", p=P))
            nc.sync.dma_start(out=moTa_sb, in_=moTa_d.rearrange("(a p) o -> p a o", p=P))
            nc.sync.dma_start(out=bw_sb, in_=bw_d.rearrange("(a p) -> p a", p=P))
            nc.sync.dma_start(out=bo2_sb, in_=bo2_d.rearrange("(a p) -> p a", p=P))
            nc.vector.memset(ones_col, 1.0)
            if is_bf16:
                nc.sync.dma_start(
                    out=xres_sb, in_=xres_d.rearrange("(a p) n -> p a n", p=P)
                )

            # ---- projections ----
            # qm[c, l] = sum_c' M[c, c'] x[c', l] + bw[c]
            for och in range(A):
                for lt in range(NLT):
                    ps = ps_st.tile([P, LT], f32, tag="st")
                    for a in range(A):
                        nc.tensor.matmul(
                            ps,
                            mT_sb[:, a, och * P:(och + 1) * P],
                            x_sb[:, a, lt * LT:(lt + 1) * LT],
                            start=(a == 0),
                            stop=(a == A - 1),
                        )
                    nc.scalar.activation(
                        out=qm_sb[:, och, lt * LT:(lt + 1) * LT],
                        in_=ps,
                        func=mybir.ActivationFunctionType.Identity,
                        bias=bw_sb[:, och:och + 1],
                    )
            # vTo[j, o] = sum_c y[c, j] MoTa[c, o] + bva[o]   (o in 0..258)
            for jc in range(JC):
                ps = ps_st.tile([P, CP], f32, tag="st")
                for a in range(A):
                    nc.tensor.matmul(
                        ps,
                        y_sb[:, a, jc * P:(jc + 1) * P],
                        moTa_sb[:, a, :],
                        start=(a == 0),
                        stop=(a == A - 1),
                    )
                nc.scalar.activation(
                    out=vTo_sb[:, jc, :],
                    in_=ps,
                    func=mybir.ActivationFunctionType.Copy,
                )

            # ---- attention, l-tile at a time ----
            for lt in range(NLT):
                lsl = slice(lt * LT, (lt + 1) * LT)
                zq0 = ps_zq.tile([P, LT], f32, tag="zq0")
                zq1 = ps_zq.tile([P, LT], f32, tag="zq1")
                zq = (zq0, zq1)
                eacc = epool.tile([P, LT], mdt, tag="eacc")
                for jc in range(JC):
                    st = ps_st.tile([P, LT], f32, tag="st")
                    if ST_FP8:
                        nc.tensor.matmul(
                            st,
                            y8_sb[:, :, jc * P:(jc + 1) * P],
                            qm_sb[:, :, lsl],
                            start=True,
                            stop=True,
                            perf_mode=mybir.MatmulPerfMode.DoubleRow,
                        )
                    else:
                        for a in range(A):
                            nc.tensor.matmul(
                                st,
                                y_sb[:, a, jc * P:(jc + 1) * P],
                                qm_sb[:, a, lsl],
                                start=(a == 0),
                                stop=(a == A - 1),
                            )
                    e_sb = epool.tile([P, LT], mdt)
                    nc.scalar.activation(
                        out=e_sb,
                        in_=st,
                        func=mybir.ActivationFunctionType.Exp,
                        scale=float(SCALE),
                    )
                    for m in range(A):
                        nc.tensor.matmul(
                            zq[m],
                            vTo_sb[:, jc, m * P:(m + 1) * P],
                            e_sb,
                            start=(jc == 0),
                            stop=(jc == JC - 1),
                        )
                    if jc == 0:
                        nc.vector.tensor_copy(out=eacc, in_=e_sb)
                    else:
                        nc.vector.tensor_add(out=eacc, in0=eacc, in1=e_sb)

                # reduce E over partitions -> denominators, then 1/den,
                # broadcast back across partitions via a DRAM round-trip
                den = ps_den.tile([1, LT], f32)
                nc.tensor.matmul(den, ones_col, eacc, start=True, stop=True)
                r_sb = rpool.tile([1, LT], f32, tag="r")
                nc.vector.reciprocal_approx_fast(out=r_sb, in_=den)
                r_dram = dpool.tile([1, LT], f32, tag="rdram")
                nc.sync.dma_start(out=r_dram, in_=r_sb)
                rbc_sb = rpool.tile([P, LT], f32, tag="rbc")
                r_bcast_ap = bass.AP(
                    tensor=r_dram.tensor,
                    offset=r_dram.offset,
                    ap=[[0, P], list(r_dram.ap[-1])],
                )
                nc.sync.dma_start(out=rbc_sb, in_=r_bcast_ap)

                # out = zq * r + bo2 + x
                for och in range(A):
                    o_sb = opool.tile([P, LT], f32)
                    nc.vector.tensor_mul(out=o_sb, in0=zq[och], in1=rbc_sb)
                    nc.vector.tensor_scalar_add(
                        out=o_sb, in0=o_sb, scalar1=bo2_sb[:, och:och + 1]
                    )
                    nc.vector.tensor_add(out=o_sb, in0=o_sb, in1=xres_sb[:, och, lsl])
                    nc.sync.dma_start(out=outr[:, och, lsl], in_=o_sb)

    nc.compile()
    return nc


_NC_CACHE = {}


def _get_nc(matmul_dt_name: str = MATMUL_DT):
    if matmul_dt_name not in _NC_CACHE:
        _NC_CACHE[matmul_dt_name] = build_nc(matmul_dt_name)
    return _NC_CACHE[matmul_dt_name]


def make_in_maps(x, y, Wq, bq, Wk, bk, Wv, bv, Wo, bo, matmul_dt_name: str = MATMUL_DT):
    f32 = np.float32
    f64 = np.float64
    if matmul_dt_name == "bfloat16":
        import ml_dtypes

        mnp = ml_dtypes.bfloat16
    else:
        mnp = np.float32
    xf = np.asarray(x, f32).reshape(B, C, N)
    yf = np.asarray(y, f32).reshape(B, C, N)
    Wq64, Wk64, Wv64, Wo64 = (np.asarray(w, f64) for w in (Wq, Wk, Wv, Wo))
    bq64, bv64, bo64 = (np.asarray(b, f64) for b in (bq, bv, bo))
    mT = np.ascontiguousarray((Wk64.T @ Wq64).T).astype(mnp)   # [c_x, c_qm]... (M^T)
    moTa = np.ascontiguousarray((Wo64 @ Wv64).T).astype(mnp)   # [c_y, o]
    bw = (Wk64.T @ bq64).astype(f32)
    bo2 = (bo64 + Wo64 @ bv64).astype(f32)
    if ST_FP8:
        import ml_dtypes as _mld
        y8 = np.clip(yf, -240, 240).astype(_mld.float8_e4m3)
    in_maps = []
    for core in range(8):
        b, h = divmod(core, 2)
        xs = np.ascontiguousarray(xf[b][:, h * NQ:(h + 1) * NQ])
        m = {
            "x": xs.astype(mnp) if mnp is not np.float32 else xs,
            "y": yf[b].astype(mnp) if mnp is not np.float32 else yf[b],
            "mT": mT, "moTa": moTa,
            "bw": bw, "bo2": bo2,
        }
        if matmul_dt_name == "bfloat16":
            m["xres"] = xs
        if ST_FP8:
            m["y8"] = y8[b]
        in_maps.append(m)
    return in_maps


def kernel(x, y, Wq, bq, Wk, bk, Wv, bv, Wo, bo):
    nc = _get_nc()
    in_maps = make_in_maps(x, y, Wq, bq, Wk, bk, Wv, bv, Wo, bo)
    res = bass_utils.run_bass_kernel_spmd(nc, in_maps, core_ids=list(range(8)))
    out = np.empty((B, C, N), np.float32)
    for core in range(8):
        b, h = divmod(core, 2)
        out[b][:, h * NQ:(h + 1) * NQ] = res.results[core]["out"]
    return out.reshape(B, C, H, W)
